# revision 50
# baseline (speedup 1.0000x reference)
"""Trainium2 Bass kernel for nn_AxialBlock (axial attention, branches W/H/T).

Self-contained: accepts FULL inputs as in reference.setup_inputs(), shards
across 8 NeuronCores as (batch x head-half), runs one SPMD Bass program,
gathers on host.

Hardcoded problem shape: x (4, 512, 16, 32, 32) f32, C=512, 8 heads, d=64.

Per-core layout: activations channel-major [C, tokens]. The work is a single
stream of 48 uniform 1024-token "units": 16 t-planes x (W branch, H branch)
then 16 h-row-pairs (T branch, combined with the W+H partial from a DRAM
scratch). Branch token orders (W: (h,w) natural; H: (w,h); T: (r,w,t)) come
from DVE reorder copies pipelined one unit ahead.

The scheduling is the point: for consecutive units, the loop emits
  [unit i-1's attention/out-proj items interleaved inside unit i's
   projection acc-groups]
so the PE sees an unbroken dense matmul stream (stays at its fast p-state)
while each unit's softmax chain (Act exp -> DVE reduce/reciprocal ->
normalize muls split Act/DVE -> DVE transpose) drains underneath the next
unit's projections. Attention per 128-token group (4 sequences x 32 tokens)
is tile_position-packed; the two concurrently-streaming PE row groups write
separate PSUM banks (h strides one bank in the score tile) -- sharing a bank
between row groups faults on hardware.
"""

import numpy as np

import concourse.bass as bass
import concourse.mybir as mybir
from concourse import bacc, tile
from concourse.bass_utils import run_bass_kernel_spmd

F32 = mybir.dt.float32
F32R = mybir.dt.float32r
BF16 = mybir.dt.bfloat16
AF = mybir.ActivationFunctionType
ALU = mybir.AluOpType

B, C, T, H, W = 4, 512, 16, 32, 32
NH, D = 8, 64
HH = 4  # heads per core (head-half)
CH = HH * D  # 256 channels per core
NEG = -30000.0


def build_nc():
    nc = bacc.Bacc("TRN2", target_bir_lowering=False, debug=False, num_devices=8)

    x_in = nc.dram_tensor("x_in", [C, T, H, W], F32, kind="ExternalInput")
    wqkv = {
        ax: nc.dram_tensor(f"wqkv_{ax}", [C, 3 * CH], F32, kind="ExternalInput")
        for ax in ("w", "h", "t")
    }
    fc = {
        ax: nc.dram_tensor(f"fc_{ax}", [CH, C], F32, kind="ExternalInput")
        for ax in ("w", "h", "t")
    }
    y_out = nc.dram_tensor("y_out", [C, T, H, W], F32, kind="ExternalOutput")
    y_wh = nc.dram_tensor("y_wh", [T, C, H * W], F32, kind="Internal")

    # T-branch pair mask over (h, c, g2, m): within a 32-token col strip the
    # two 16-token sequences must not attend to each other.
    mrows = np.arange(128) % 32
    mcols = np.arange(32)
    m2 = np.where((mrows[:, None] // 16) == (mcols[None, :] // 16), 0.0, NEG)
    mask_np = np.broadcast_to(
        m2[:, None, None, None, :], (128, 2, 2, 2, 32)
    ).astype(np.float32)
    mask_dram = nc.inline_tensor(np.ascontiguousarray(mask_np), name="tmask")

    with tile.TileContext(nc) as tc:
        with (
            tc.tile_pool(name="consts", bufs=1) as consts,
            tc.tile_pool(name="xtp", bufs=2) as xtp,
            tc.tile_pool(name="qkv", bufs=2) as qkvp,
            tc.tile_pool(name="att", bufs=4) as attp,
            tc.tile_pool(name="yp", bufs=2) as yp,
            tc.tile_pool(name="ps", bufs=1, space="PSUM") as ps,
        ):
            w_t = {}
            fc_t = {}
            for ax in ("w", "h", "t"):
                w_t[ax] = consts.tile([128, 4, 3 * CH], BF16, name=f"w_{ax}")
                nc.gpsimd.dma_start(
                    out=w_t[ax],
                    in_=wqkv[ax].rearrange("(kc kp) m -> kp kc m", kp=128),
                )
                fc_t[ax] = consts.tile([128, 2, C], F32R, name=f"fc_{ax}")
                nc.sync.dma_start(
                    out=fc_t[ax],
                    in_=fc[ax].rearrange("(kc kp) m -> kp kc m", kp=128)
                    .bitcast(F32R),
                )
            mask_t = consts.tile([128, 2, 2, 2, 32], F32, name="mask_t")
            nc.sync.dma_start(out=mask_t, in_=mask_dram[:, :, :, :, :])

            def qk_mms(xv, ax, ntok, qt, kt, fill):
                """q/k projection. xv: callable(ic, tt) -> rhs AP of 512
                tokens in the branch token order. Writes qt/kt bf16.
                fill() runs one interleaved attention item per acc-group."""
                for tt in range(ntok // 512):
                    for mc in range(4):  # q0 q1 k0 k1
                        dst = qt if mc < 2 else kt
                        oc = mc % 2
                        acc = ps.tile([128, 512], F32, name="acc", tag="big",
                                      bufs=2)
                        for ic in range(4):
                            nc.tensor.matmul(
                                acc,
                                w_t[ax][:, ic, mc * 128 : (mc + 1) * 128],
                                xv(ic, tt),
                                start=(ic == 0),
                                stop=(ic == 3),
                            )
                        nc.scalar.copy(
                            out=dst[:, oc, tt * 512 : (tt + 1) * 512], in_=acc
                        )
                        fill()

            def v_mms(xg, ax, ntok, vr, v_copy, fill):
                """v projection, token-major. xg: callable(ic, g) -> lhsT AP
                [128, 128] = tokens g*128..+128 in branch order."""
                for g in range(ntok // 128):
                    acc2 = ps.tile([128, 256], F32, name="acc2", tag="big",
                                   bufs=2)
                    for ic in range(4):
                        nc.tensor.matmul(
                            acc2,
                            xg(ic, g),
                            w_t[ax][:, ic, 512:768],
                            start=(ic == 0),
                            stop=(ic == 3),
                        )
                    v_copy(vr[:, g, :], acc2)
                    fill()

            def score_mms(g, sct, qt, kt):
                # sct [128, 2(h), 2(c), 8(g), 32(m)]: h strides one full PSUM
                # bank, so the two concurrently-streaming tile_position row
                # groups never share a bank (same constraint the per-h tiles
                # of the baseline satisfied).
                for s in range(4):
                    q32 = slice(g * 128 + s * 32, g * 128 + (s + 1) * 32)
                    for c in range(2):
                        for h in range(2):
                            nc.tensor.matmul(
                                sct[s * 32 : (s + 1) * 32, h, c, g, :],
                                qt[h * 64 : (h + 1) * 64, c, q32],
                                kt[h * 64 : (h + 1) * 64, c, q32],
                                start=True,
                                stop=True,
                                tile_position=(h * 64, s * 32),
                                skip_group_check=True,
                            )
            def softmax_pair(g, sct, masked):
                """Softmax for groups g, g+1 batched: one mask add, one exp,
                one reduce, one reciprocal (all <=3 free dims). The normalize
                muls (split Act/DVE) reorder into a canonical (c, h, m) tile
                per group that feeds the transpose. Returns (attT_g, attT_g1).
                """
                scv = sct[:, :, :, g : g + 2, :].rearrange(
                    "p h c g m -> p h c (g m)"
                )
                if masked:
                    nc.vector.tensor_tensor(
                        out=scv, in0=scv,
                        in1=mask_t.rearrange("p h c g m -> p h c (g m)"),
                        op=ALU.add,
                    )
                # e2 memory layout (h, c, g2, m) == the exp input order
                e2 = attp.tile([128, 2, 2, 2, 32], BF16, name="e2",
                               tag="e2", bufs=6)
                nc.scalar.activation(
                    out=e2.rearrange("p h c g m -> p h c (g m)"),
                    in_=scv,
                    func=AF.Exp,
                )
                rs = attp.tile([128, 8], F32, name="rs", tag="rs", bufs=6)
                nc.vector.tensor_reduce(
                    out=rs,
                    in_=e2.rearrange("p h c g m -> p (h c g) m"),
                    axis=mybir.AxisListType.X,
                    op=ALU.add,
                )
                rv = attp.tile([128, 8], F32, name="rv", tag="rv", bufs=6)
                nc.vector.reciprocal(out=rv, in_=rs)
                attTs = []
                for gj in range(2):
                    attn = attp.tile([128, 2, 2, 32], BF16, name="attn",
                                     tag="attn", bufs=6)
                    for c in range(2):
                        for h in range(2):
                            ri = h * 4 + c * 2 + gj
                            if c == 0:
                                nc.scalar.mul(
                                    out=attn[:, c, h, :],
                                    in_=e2[:, h, c, gj, :],
                                    mul=rv[:, ri : ri + 1],
                                )
                            else:
                                nc.vector.tensor_scalar_mul(
                                    out=attn[:, c, h, :],
                                    in0=e2[:, h, c, gj, :],
                                    scalar1=rv[:, ri : ri + 1],
                                )
                    attT = attp.tile([128, 2, 2, 32], BF16, name="attT",
                                     tag="attT", bufs=16)
                    nc.vector.transpose(
                        out=attT.rearrange("p c h n -> p (c h n)"),
                        in_=attn.rearrange("p c h n -> p (c h n)"),
                    )
                    attTs.append(attT)
                return attTs

            def av_mms(g, vr, attT, ot, all_act=False):
                for s in range(4):
                    avt = ps.tile([128, 2, 32], F32, name=f"av{s % 2}",
                                  tag=f"av{s % 2}", bufs=2)
                    for c in range(2):
                        for h in range(2):
                            nc.tensor.matmul(
                                avt[h * 64 : (h + 1) * 64, c, :],
                                vr[s * 32 : (s + 1) * 32, g,
                                   (2 * c + h) * 64 : (2 * c + h + 1) * 64],
                                attT[s * 32 : (s + 1) * 32, c, h, :],
                                start=True,
                                stop=True,
                                tile_position=(s * 32, h * 64),
                                skip_group_check=True,
                            )
                    dst = ot[:, :, g * 128 + s * 32 : g * 128 + (s + 1) * 32]
                    if s < 2 or all_act:
                        nc.scalar.copy(out=dst, in_=avt)
                    else:
                        nc.vector.tensor_copy(out=dst, in_=avt)

            def op_item(ax, ot, tt, oc, write_fn):
                yps = ps.tile([128, 512], F32, name="yps", tag="big",
                              bufs=2)
                for ic in range(2):
                    nc.tensor.matmul(
                        yps,
                        fc_t[ax][:, ic, oc * 128 : (oc + 1) * 128],
                        ot[:, ic, tt * 512 : (tt + 1) * 512],
                        start=(ic == 0),
                        stop=(ic == 1),
                    )
                write_fn(oc, tt, yps)

            def branch_front(ax, ntok, xv, xg, v_copy, fillers):
                """Projections for one branch, with the previous unit's
                attention items (score packs, AV packs) interleaved between
                acc-groups so the PE never sees a sparse stretch."""
                ng = ntok // 128
                qt = qkvp.tile([128, 2, 1024], BF16, name="qt", tag="qt",
                               bufs=2)[:, :, :ntok]
                kt = qkvp.tile([128, 2, 1024], BF16, name="kt", tag="kt",
                               bufs=2)[:, :, :ntok]
                vr = qkvp.tile([128, 8, 256], BF16, name="vr", tag="vr",
                               bufs=2)[:, :ng, :]
                ot = qkvp.tile([128, 2, 1024], F32R, name="ot", tag="ot",
                               bufs=2)[:, :, :ntok]
                it = iter(fillers)
                state = {"skip": 0}

                def fill():
                    if state["skip"] > 0:  # let qt/kt copies get ahead
                        state["skip"] -= 1
                        return
                    f = next(it, None)
                    if f is not None:
                        f()

                qk_mms(xv, ax, ntok, qt, kt, fill)
                v_mms(xg, ax, ntok, vr, v_copy, fill)
                for f in it:
                    f()
                return (ax, ntok, qt, kt, vr, ot)

            def attn_fillers(st, masked, write_fn):
                """Interleavable attention + out-projection items for a unit:
                score packs, AV packs, and (tt, oc) out-proj chunks ordered so
                each item's dependencies were issued several items earlier."""
                ax, ntok, qt, kt, vr, ot = st
                ng = ntok // 128
                sct = ps.tile([128, 2, 2, 8, 32], F32, name="sc", tag="sc",
                              bufs=1)
                attTs = [None] * ng
                items = []

                def sc_item(gp):
                    score_mms(gp, sct, qt, kt)
                    score_mms(gp + 1, sct, qt, kt)
                    attTs[gp], attTs[gp + 1] = softmax_pair(gp, sct, masked)

                def av_item(g):
                    av_mms(g, vr, attTs[g], ot, all_act=(ax != "w"))

                for gp in range(0, ng, 2):
                    items.append(lambda gp=gp: sc_item(gp))
                for g in range(ng // 2):
                    items.append(lambda g=g: av_item(g))
                for k in range(ng // 2):
                    items.append(lambda g=ng // 2 + k: av_item(g))
                    items.append(
                        lambda oc=k: op_item(ax, ot, 0, oc, write_fn))
                for oc in range(4):
                    items.append(lambda oc=oc: op_item(ax, ot, 1, oc,
                                                       write_fn))
                return items

            def vcp_vec(out, in_):
                nc.vector.tensor_copy(out=out, in_=in_)

            def vcp_act(out, in_):
                nc.scalar.copy(out=out, in_=in_)

            # ---------------- Phase 1: W + H branches per t-plane
            def load_x(p):
                xt = xtp.tile([128, 4, 1024], BF16, name="xt", tag="xt",
                              bufs=3)
                for cc in range(4):
                    nc.gpsimd.dma_start(
                        out=xt[:, cc, :],
                        in_=x_in[cc * 128 : (cc + 1) * 128, p, :, :]
                        .rearrange("p h w -> p (h w)"),
                    )
                return xt

            def make_xth(xt):
                # w-major reorder (GpSimd, which is otherwise idle),
                # pipelined one plane ahead so the copy's input DMA has
                # already landed when it issues (no head-of-line blocking)
                xth = xtp.tile([128, 4, 1024], BF16, name="xth", tag="xth",
                               bufs=3)
                nc.vector.tensor_copy(
                    out=xth.rearrange("p c (w h) -> p c w h", h=32),
                    in_=xt.rearrange("p c (h w) -> p c w h", w=32),
                )
                return xth

            # ---------------- Phase 2 helpers: T branch on ROW PAIRS
            # (two adjacent h-rows = 1024 tokens, same shape as a plane)
            def load_xn(j):
                r = 2 * j
                xn = xtp.tile([128, 4, 1024], BF16, name="xn", tag="xt",
                              bufs=3)
                for cc in range(4):
                    nc.gpsimd.dma_start(
                        out=xn[:, cc, :].rearrange(
                            "p (t r w) -> p t (r w)", r=2, w=32),
                        in_=x_in[cc * 128 : (cc + 1) * 128, :, r : r + 2, :]
                        .rearrange("p t r w -> p t (r w)"),
                    )
                return xn

            def make_xtt(xn):
                # per row: (w, t) reorder; rows stay in separate halves
                xtt = xtp.tile([128, 4, 1024], BF16, name="xtt", tag="xth",
                               bufs=3)
                nc.vector.tensor_copy(
                    out=xtt.rearrange("p c (r w t) -> p c r w t", r=2, w=32),
                    in_=xn.rearrange("p c (t r w) -> p c r w t", r=2, w=32),
                )
                return xtt

            def load_ywh(j):
                r = 2 * j
                ywh = yp.tile([128, 4, 1024], F32, name="ywh", tag="ywh",
                              bufs=2)
                for cc in range(4):
                    nc.sync.dma_start(
                        out=ywh[:, cc, :].rearrange(
                            "p (t rw) -> p t rw", rw=64),
                        in_=y_wh[:, cc * 128 : (cc + 1) * 128,
                                 r * 32 : (r + 2) * 32].rearrange(
                                     "t p rw -> p t rw"),
                    )
                return ywh

            # ---------------- Unit stream driver: every unit (W-plane,
            # H-plane, T row-pair) is [front: projections] [attn: scores +
            # softmax] [back: AV + out-projection]. The loop runs
            # attn(i-1), front(i), back(i-1) so each unit's softmax chains
            # drain underneath the next unit's ~10us of big matmuls.
            xts = {0: load_x(0), 1: load_x(1)}
            xths = {0: make_xth(xts[0])}
            xns = {}
            xtts = {}
            ywhs = {}

            def units():
                for p in range(T):
                    if p + 2 < T:
                        xts[p + 2] = load_x(p + 2)
                    if p + 1 < T:
                        xths[p + 1] = make_xth(xts[p + 1])
                    if p == T - 1:
                        # phase-2 prologue: issue early so pair-0 inputs
                        # land while plane 15 computes
                        xns[0] = load_xn(0)
                        xns[1] = load_xn(1)
                        xtts[0] = make_xtt(xns[0])
                    xt = xts.pop(p)
                    xth = xths.pop(p)
                    ysb = yp.tile([128, 4, 1024], F32, name="ysb",
                                  tag="ysb", bufs=2)

                    def xv_w(ic, tt, xt=xt):
                        return xt[:, ic, tt * 512 : (tt + 1) * 512]

                    def xg_w(ic, g, xt=xt):
                        return xt[:, ic, g * 128 : (g + 1) * 128]

                    def xv_h(ic, tt, xth=xth):
                        return xth[:, ic, tt * 512 : (tt + 1) * 512]

                    def xg_h(ic, g, xth=xth):
                        return xth[:, ic, g * 128 : (g + 1) * 128]

                    def wr_w(oc, tt, yps, ysb=ysb):
                        dst = ysb[:, oc, tt * 512 : (tt + 1) * 512]
                        if oc < 2:
                            nc.scalar.copy(out=dst, in_=yps)
                        else:
                            nc.vector.tensor_copy(out=dst, in_=yps)

                    def wr_h(oc, tt, yps, ysb=ysb):
                        dv = ysb[:, oc, :].rearrange(
                            "p (h w) -> p w h", w=32
                        )[:, 16 * tt : 16 * (tt + 1), :]
                        nc.vector.tensor_tensor(
                            out=dv,
                            in0=yps.rearrange("p (w h) -> p w h", h=32),
                            in1=dv,
                            op=ALU.add,
                        )

                    def post_h(p=p, ysb=ysb):
                        for cc in range(4):
                            nc.sync.dma_start(
                                out=y_wh[p, cc * 128 : (cc + 1) * 128, :],
                                in_=ysb[:, cc, :],
                            )
                        if p == T - 1:
                            # pair-0 scratch read must be issued after the
                            # final scratch write above (program order)
                            ywhs[0] = load_ywh(0)

                    yield ("w", xv_w, xg_w, False, wr_w, None)
                    yield ("h", xv_h, xg_h, False, wr_h, post_h)

                for j in range(H // 2):
                    if j + 2 < H // 2:
                        xns[j + 2] = load_xn(j + 2)
                    if j + 1 < H // 2:
                        xtts[j + 1] = make_xtt(xns[j + 1])
                    if j >= 1:
                        ywhs[j] = load_ywh(j)
                    xns.pop(j, None)
                    xtt = xtts.pop(j)
                    ysb = yp.tile([128, 4, 1024], F32, name="ysb2",
                                  tag="ysb", bufs=2)

                    def xv_t(ic, tt, xtt=xtt):
                        return xtt[:, ic, tt * 512 : (tt + 1) * 512]

                    def xg_t(ic, g, xtt=xtt):
                        return xtt[:, ic, g * 128 : (g + 1) * 128]

                    def wr2(oc, tt, yps, ysb=ysb, j=j):
                        # yps free order (w, t) for row tt; ysb natural
                        # (t, r2, w); ywh looked up late (pair 0 loads in
                        # post_h(15))
                        ywh = ywhs[j]
                        dst = ysb[:, oc, :].rearrange(
                            "p (t r w) -> p r w t", r=2, w=32)[:, tt, :, :]
                        nc.vector.tensor_tensor(
                            out=dst,
                            in0=yps.rearrange("p (w t) -> p w t", t=16),
                            in1=ywh[:, oc, :].rearrange(
                                "p (t r w) -> p r w t", r=2, w=32
                            )[:, tt, :, :],
                            op=ALU.add,
                        )

                    def post_t(j=j, ysb=ysb):
                        r = 2 * j
                        for cc in range(4):
                            nc.sync.dma_start(
                                out=y_out[cc * 128 : (cc + 1) * 128, :,
                                          r : r + 2, :]
                                .rearrange("p t r w -> p t (r w)"),
                                in_=ysb[:, cc, :].rearrange(
                                    "p (t rw) -> p t rw", rw=64),
                            )

                    yield ("t", xv_t, xg_t, True, wr2, post_t)

            pend = None
            for ax, xv, xg, masked, write_fn, post in units():
                fillers = (attn_fillers(pend[0], pend[1], pend[2])
                           if pend is not None else [])
                st = branch_front(ax, 1024, xv, xg, vcp_vec, fillers)
                if pend is not None and pend[3] is not None:
                    pend[3]()
                pend = (st, masked, write_fn, post)
            for f in attn_fillers(pend[0], pend[1], pend[2]):
                f()
            if pend[3] is not None:
                pend[3]()
    nc.compile()
    return nc


_NC_CACHE = {}


def _get_nc():
    if "nc" not in _NC_CACHE:
        _NC_CACHE["nc"] = build_nc()
    return _NC_CACHE["nc"]


def kernel(x, wq_w, wk_w, wv_w, fc_w, fb_w,
           wq_h, wk_h, wv_h, fc_h, fb_h,
           wq_t, wk_t, wv_t, fc_t, fb_t, _trace=False):
    x = np.asarray(x, np.float32)
    scale = 1.0 / np.sqrt(np.float32(D))
    branches = {
        "w": (np.asarray(wq_w, np.float32), np.asarray(wk_w, np.float32),
              np.asarray(wv_w, np.float32), np.asarray(fc_w, np.float32)),
        "h": (np.asarray(wq_h, np.float32), np.asarray(wk_h, np.float32),
              np.asarray(wv_h, np.float32), np.asarray(fc_h, np.float32)),
        "t": (np.asarray(wq_t, np.float32), np.asarray(wk_t, np.float32),
              np.asarray(wv_t, np.float32), np.asarray(fc_t, np.float32)),
    }
    fb_sum = (np.asarray(fb_w, np.float32) + np.asarray(fb_h, np.float32)
              + np.asarray(fb_t, np.float32))

    in_maps = []
    for core in range(8):
        b, hh = core // 2, core % 2
        m = {"x_in": np.ascontiguousarray(x[b])}
        cols = slice(hh * CH, (hh + 1) * CH)
        for ax, (wq, wk, wv, fcm) in branches.items():
            m[f"wqkv_{ax}"] = np.ascontiguousarray(
                np.concatenate(
                    [wq[:, cols] * scale, wk[:, cols], wv[:, cols]], axis=1
                )
            )
            m[f"fc_{ax}"] = np.ascontiguousarray(fcm[cols, :])
        in_maps.append(m)

    nc = _get_nc()
    res = run_bass_kernel_spmd(
        nc, in_maps, core_ids=list(range(8)), trace=_trace,
    )
    y = np.empty((B, C, T, H, W), np.float32)
    for b in range(B):
        y[b] = res.results[2 * b]["y_out"] + res.results[2 * b + 1]["y_out"]
    y += fb_sum[None, :, None, None, None]
    if _trace:
        _NC_CACHE["last_result"] = res
    return y
